# revision 16
# baseline (speedup 1.0000x reference)
"""MultiPropMLP (MoE-routed tiny MLP) Trainium2 kernel — expert-sharded.

Problem: out[n] = MLP_{idx[n]}(xs[n]) for N = 8192*128 samples, K = 8 experts,
MLP = 16 -> 64 -> relu -> 64 -> relu -> 1 with per-expert weights.

Sharding: expert-parallel across the 8 NeuronCores (K == n_cores). The host
routes each sample to the core owning its expert (stable argsort of idxs, a
pure sharding/layout step), so every core runs ONE dense 3-layer MLP over
~N/8 samples — no on-device routing, masking, gather, or idx tensor at all.
This removes the 8x all-K overcompute of the data-parallel formulation (the
previous 982us kernel): PE work drops from ~12 to 1.5 cycles/sample and the
PSUM-evacuation volume drops 8x.

Per-core layout (host-packed, feature-major, 2 samples per matmul column):
  tile t (1024 samples) = xt [32, 512]: col c holds sample 2c in partitions
  0-15 (features) and sample 2c+1 in partitions 16-31. Tiles are stacked 2
  deep across partitions (matmul operand base partitions are limited to
  {0,32,64}) -> DRAM xs2 [64, (CT/2)*512]; one [64, 8*512] slab DMA (gpsimd,
  the only engine allowed to cast f32->f32r) feeds 16 tiles. Per tile (all
  matmuls float32r, 512-col moving => 1 cyc/row):
    h0 [128,512] = relu(w0bd.T @ xt + b0)    w0bd = diag(W0, W0) [32, 128]
                                             (replicated at partitions
                                             0/32/64/96 to satisfy the
                                             lhsT/rhs same-base-partition rule)
    h1 [128,512] = relu(w1bd.T @ h0 + b1)    w1bd = diag(W1, W1) [128, 128]
    l2 [64, 512] += w2s_j.T @ h1             w2s_j [128, 64]: zero except col
                                             2j (rows :64) and 2j+1 (rows 64:)
                                             = W2, j = t % 32: 32 tiles
                                             accumulate into one PSUM block,
                                             amortizing the tiny-output evac.
  Engine balance per tile: PE 3x213ns; ACT h0 evac (relu+bias, 612ns); DVE
  h1 evac as [128,1024] pairs (596ns/tile); l2 block copy on DVE every 32
  tiles. b2 is folded in on the host during unpermute.

Note: walrus in this toolchain accepts only ONE sync-wait per instruction;
_split_ctrl_waits() hoists Tile's multi-waits onto single-wait nops.
"""

import numpy as np

R, S, D_IN, WIDTH, K = 8192, 128, 16, 64, 8
N = R * S
NCORES = 8
TILE = 512          # moving columns per matmul tile (= 1024 samples)
LBLK = 32           # tiles accumulated per l2 PSUM block
SLAB = 8            # [64,512] blocks (16 tiles) per xs DMA slab
LAG = 8             # tile-pairs the l2 matmuls trail the L0/L1 stream by

_cache = {}


def _slab_sizes(NV):
    """DMA slab sizes in [64,512] blocks: small first so compute starts
    early, then full SLAB-sized."""
    sizes = []
    for sz in (1, 1, 2, 4):
        if sum(sizes) >= NV:
            break
        sizes.append(min(sz, NV - sum(sizes)))
    while sum(sizes) < NV:
        sizes.append(min(SLAB, NV - sum(sizes)))
    return sizes


def _build_nc(CT):
    import concourse.bass as bass
    import concourse.mybir as mybir
    from concourse import tile

    f32 = mybir.dt.float32
    f32r = mybir.dt.float32r
    NV = CT // 2                     # [64, 512] 2-tile blocks
    LB = -(-CT // LBLK)              # l2 blocks
    sizes = _slab_sizes(NV)
    NS = len(sizes)
    slab_first_tile = []
    acc = 0
    for sz in sizes:
        slab_first_tile.append(2 * acc)
        acc += sz
    slab_off = [sum(sizes[:w]) for w in range(NS)]
    nc = bass.Bass()

    xs2 = nc.dram_tensor("xs2", [64, NV * TILE], f32, kind="ExternalInput")
    w0st = nc.dram_tensor("w0st", [64, 128], f32, kind="ExternalInput")
    w1bd = nc.dram_tensor("w1bd", [128, 128], f32, kind="ExternalInput")
    w2bk = nc.dram_tensor("w2bk", [128, LBLK * 64], f32, kind="ExternalInput")
    b0bd = nc.dram_tensor("b0bd", [128, 1], f32, kind="ExternalInput")
    b1bd = nc.dram_tensor("b1bd", [128, 1], f32, kind="ExternalInput")
    out_c = nc.dram_tensor("out_c", [64, LB * TILE], f32, kind="ExternalOutput")

    with tile.TileContext(nc) as tc:
        with (
            tc.tile_pool(name="const", bufs=1) as cpool,
            tc.tile_pool(name="xt", bufs=4) as xtpool,
            tc.tile_pool(name="h0sb", bufs=3) as h0pool,
            tc.tile_pool(name="h1sb", bufs=LAG + 3) as h1pool,
            tc.tile_pool(name="outsb", bufs=1) as opool,
            tc.tile_pool(name="ps_h0", bufs=2, space="PSUM") as ps_h0,
            tc.tile_pool(name="ps_h1", bufs=2, space="PSUM") as ps_h1,
            tc.tile_pool(name="ps_l2", bufs=2, space="PSUM") as ps_l2,
        ):
            # consts via SP/HWDGE as f32 (gpsimd SWDGE is reserved for the
            # xs slabs; only gpsimd DMAs may cast, so convert on idle
            # engines instead)
            w0f = cpool.tile([64, 128], f32, tag="w0f")
            nc.sync.dma_start(w0f[:], w0st[:])
            b0_sb = cpool.tile([128, 1], f32, tag="b0")
            nc.sync.dma_start(b0_sb[:], b0bd[:])
            w1f = cpool.tile([128, 128], f32, tag="w1f")
            nc.sync.dma_start(w1f[:], w1bd[:])
            b1_sb = cpool.tile([128, 1], f32, tag="b1")
            nc.sync.dma_start(b1_sb[:], b1bd[:])
            w2f = cpool.tile([128, LBLK * 64], f32, tag="w2f")
            nc.sync.dma_start(w2f[:], w2bk[:])
            w0_sb = cpool.tile([64, 128], f32r, tag="w0")
            nc.vector.tensor_copy(w0_sb[:], w0f[:])
            w1_sb = cpool.tile([128, 128], f32r, tag="w1")
            nc.vector.tensor_copy(w1_sb[:], w1f[:])
            w2_sb = cpool.tile([128, LBLK * 64], f32r, tag="w2")
            nc.gpsimd.tensor_copy(w2_sb[:], w2f[:])

            out_sb = opool.tile([64, LB * TILE], f32, tag="out")

            slabs = [None] * NS

            def issue_slab(w):
                if w >= NS or slabs[w] is not None:
                    return
                bs = sizes[w]
                slabs[w] = xtpool.tile(
                    [64, SLAB * TILE], f32r, tag="xt", name="slab")
                nc.gpsimd.dma_start(
                    slabs[w][:, : bs * TILE],
                    xs2[:, TILE * slab_off[w] : TILE * (slab_off[w] + bs)],
                )

            for w in range(min(3, NS)):
                issue_slab(w)

            def emit_l2(pr):
                """L2 matmuls for tile pair pr (tiles 2pr, 2pr+1), plus the
                block evac + out-chunk DMA at block boundaries."""
                h1, _ = pending_h1[pr]
                for tt in (2 * pr, 2 * pr + 1):
                    b, j = divmod(tt, LBLK)
                    if j == 0:
                        l2state[0] = ps_l2.tile(
                            [64, TILE], f32, tag="l2", name="l2ps")
                    last = j == LBLK - 1 or tt == CT - 1
                    nc.tensor.matmul(
                        l2state[0][:], w2_sb[:, 64 * j : 64 * (j + 1)],
                        h1[:, TILE * (tt % 2) : TILE * (tt % 2 + 1)],
                        start=(j == 0), stop=last,
                    )
                    if last:
                        nc.vector.tensor_copy(
                            out_sb[:, TILE * b : TILE * (b + 1)], l2state[0][:]
                        )
                        nc.sync.dma_start(
                            out_c[:, TILE * b : TILE * (b + 1)],
                            out_sb[:, TILE * b : TILE * (b + 1)],
                        )

            pending_h1 = {}
            l2state = [None]
            h1ps = None
            wslab = 0
            for t in range(CT):
                v, s = divmod(t, 2)
                if v >= slab_off[wslab] + sizes[wslab]:
                    wslab += 1
                vv = v - slab_off[wslab]
                if s == 0 and vv == 0:
                    issue_slab(wslab + 3)
                slab = slabs[wslab]
                h0ps = ps_h0.tile([128, TILE], f32, tag="h0ps")
                nc.tensor.matmul(
                    h0ps[:], w0_sb[32 * s : 32 * (s + 1), :],
                    slab[32 * s : 32 * (s + 1), TILE * vv : TILE * (vv + 1)],
                    start=True, stop=True,
                )
                h0 = h0pool.tile([128, TILE], f32r, tag="h0")
                nc.scalar.activation(
                    h0[:], h0ps[:], mybir.ActivationFunctionType.Relu,
                    bias=b0_sb[:, 0:1],
                )
                p = t % 2
                if p == 0:
                    h1ps = ps_h1.tile([128, 2 * TILE], f32, tag="h1ps")
                nc.tensor.matmul(
                    h1ps[:, TILE * p : TILE * (p + 1)], w1_sb[:], h0[:],
                    start=True, stop=True,
                )
                if p == 1:
                    h1 = h1pool.tile([128, 2 * TILE], f32r, tag="h1")
                    nc.vector.tensor_scalar(
                        h1[:], h1ps[:], b1_sb[:, 0:1], 0.0,
                        mybir.AluOpType.add, mybir.AluOpType.max,
                    )
                    pr = v
                    pending_h1[pr] = (h1, t)
                    # lag l2 so the (late-loaded) w2 stationary never stalls
                    # PE's in-order stream; catch up once it has arrived
                    target = LAG if pr < LAG + 4 else max(0, LAG - (pr - LAG - 3))
                    while pending_h1 and min(pending_h1) <= pr - target:
                        pq = min(pending_h1)
                        emit_l2(pq)
                        del pending_h1[pq]
            for pq in sorted(pending_h1):
                emit_l2(pq)

    _split_ctrl_waits(nc, mybir)
    return nc


def _split_ctrl_waits(nc, mybir):
    """walrus in this container accepts only one sync-wait per instruction;
    Tile attaches one wait per dependency lane. Hoist extras onto preceding
    single-wait nops on the same engine (equivalent ordering semantics)."""
    for bb in nc.main_func.blocks:
        newlist = []
        changed = False
        for ins in bb.instructions:
            si = ins.sync_info
            if si is not None and len(si.on_wait) > 1:
                waits = list(si.on_wait)
                for j, w in enumerate(waits[:-1]):
                    nop = mybir.InstNoOp(name=f"{ins.name}-wsplit-{j}", ins=[], outs=[])
                    nop.engine = ins.engine
                    nop.sync_info = mybir.SyncInfo(on_wait=[w], on_update=[])
                    newlist.append(nop)
                si.on_wait = [waits[-1]]
                ins.sync_info = si
                changed = True
            newlist.append(ins)
        if changed:
            bb.instructions = newlist
    return nc


def _prep_core_consts(W0k, b0k, W1k, b1k, W2k):
    f = np.float32
    w0bd = np.zeros((32, 128), f)
    w0bd[:16, :64] = W0k
    w0bd[16:, 64:] = W0k
    w0st = np.tile(w0bd, (2, 1))                      # [64, 128]
    w1 = np.zeros((128, 128), f)
    w1[:64, :64] = W1k
    w1[64:, 64:] = W1k
    w2bk = np.zeros((128, LBLK * 64), f)
    for j in range(LBLK):
        w2bk[:64, 64 * j + 2 * j] = W2k[:, 0]
        w2bk[64:, 64 * j + 2 * j + 1] = W2k[:, 0]
    b0 = np.concatenate([b0k, b0k]).astype(f).reshape(128, 1)
    b1 = np.concatenate([b1k, b1k]).astype(f).reshape(128, 1)
    return dict(w0st=w0st, w1bd=w1, w2bk=w2bk, b0bd=b0, b1bd=b1)


def _pack_xs(xs_k, CT):
    """[count, 16] -> [64, (CT/2)*512]; see module docstring."""
    NV = CT // 2
    X = np.zeros((CT * 1024, D_IN), np.float32)
    X[: len(xs_k)] = xs_k
    A = X.reshape(NV, 2, TILE, 2, D_IN)               # [v, s, c, o, f]
    return np.ascontiguousarray(
        A.transpose(1, 3, 4, 0, 2).reshape(64, NV * TILE))


def kernel(idxs, xs, W0, b0, W1, b1, W2, b2):
    from concourse.bass_utils import run_bass_kernel_spmd

    idx_flat = np.asarray(idxs).reshape(N)
    xs_flat = np.ascontiguousarray(np.asarray(xs, np.float32).reshape(N, D_IN))
    W0, b0 = np.asarray(W0, np.float32), np.asarray(b0, np.float32)
    W1, b1 = np.asarray(W1, np.float32), np.asarray(b1, np.float32)
    W2, b2 = np.asarray(W2, np.float32), np.asarray(b2, np.float32)

    order = np.argsort(idx_flat, kind="stable")
    counts = np.bincount(idx_flat, minlength=K)
    starts = np.zeros(K + 1, np.int64)
    starts[1:] = np.cumsum(counts)

    CT = max(2, -(-int(counts.max()) // 1024))
    CT = -(-CT // 2) * 2                              # multiple of 2
    if CT not in _cache:
        _cache[CT] = _build_nc(CT)
        _cache["nc"] = _cache[CT]                     # for test.py's TimelineSim
    nc = _cache[CT]
    LB = -(-CT // LBLK)

    in_maps = []
    perms = []
    for c in range(NCORES):
        perm_k = order[starts[c] : starts[c + 1]]
        perms.append(perm_k)
        in_maps.append(dict(
            xs2=_pack_xs(xs_flat[perm_k], CT),
            **_prep_core_consts(W0[c], b0[c], W1[c], b1[c], W2[c]),
        ))

    res = run_bass_kernel_spmd(nc, in_maps, list(range(NCORES))).results
    out = np.empty(N, np.float32)
    for c in range(NCORES):
        oc = res[c]["out_c"].reshape(32, 2, LB, TILE)
        vals = oc.transpose(2, 0, 3, 1).reshape(-1)[: counts[c]]
        out[perms[c]] = vals + b2[c, 0]
    return out.reshape(R, S, 1)


# revision 27
# speedup vs baseline: 1.1386x; 1.1386x over previous
"""MultiPropMLP (MoE-routed tiny MLP) Trainium2 kernel — expert-sharded.

Problem: out[n] = MLP_{idx[n]}(xs[n]) for N = 8192*128 samples, K = 8 experts,
MLP = 16 -> 64 -> relu -> 64 -> relu -> 1 with per-expert weights.

Sharding: expert-parallel across the 8 NeuronCores (K == n_cores). The host
routes each sample to the core owning its expert (stable argsort of idxs, a
pure sharding/layout step), so every core runs ONE dense 3-layer MLP over
~N/8 samples — no on-device routing, masking, gather, or idx tensor at all.
This removes the 8x all-K overcompute of the data-parallel formulation (the
previous 982us kernel): PE work drops from ~12 to 1.5 cycles/sample and the
PSUM-evacuation volume drops 8x.

Per-core layout (host-packed, feature-major, 2 samples per matmul column):
  tile t (1024 samples) = xt [32, 512]: col c holds sample 2c in partitions
  0-15 (features) and sample 2c+1 in partitions 16-31. Tiles are stacked 2
  deep across partitions (matmul operand base partitions are limited to
  {0,32,64}) -> DRAM xs2 [64, (CT/2)*512]; one [64, 8*512] slab DMA (gpsimd,
  the only engine allowed to cast f32->f32r) feeds 16 tiles. Per tile (all
  matmuls float32r, 512-col moving => 1 cyc/row):
    h0 [128,512] = relu(w0bd.T @ xt + b0)    w0bd = diag(W0, W0) [32, 128]
                                             (replicated at partitions
                                             0/32/64/96 to satisfy the
                                             lhsT/rhs same-base-partition rule)
    h1 [128,512] = relu(w1bd.T @ h0 + b1)    w1bd = diag(W1, W1) [128, 128]
    l2 [64, 512] += w2s_j.T @ h1             w2s_j [128, 64]: zero except col
                                             2j (rows :64) and 2j+1 (rows 64:)
                                             = W2, j = t % 32: 32 tiles
                                             accumulate into one PSUM block,
                                             amortizing the tiny-output evac.
  Engine balance per tile: PE 3x213ns; ACT h0 evac (relu+bias, 612ns); DVE
  h1 evac as [128,1024] pairs (596ns/tile); l2 block copy on DVE every 32
  tiles. b2 is folded in on the host during unpermute.

Note: walrus in this toolchain accepts only ONE sync-wait per instruction;
_split_ctrl_waits() hoists Tile's multi-waits onto single-wait nops.
"""

import numpy as np

R, S, D_IN, WIDTH, K = 8192, 128, 16, 64, 8
N = R * S
NCORES = 8
TILE = 512          # moving columns per matmul tile (= 1024 samples)
LBLK = 32           # tiles accumulated per l2 PSUM block
SLAB = 8            # [64,512] blocks (16 tiles) per xs DMA slab
LAG = 8             # tile-pairs the l2 matmuls trail the L0/L1 stream by

_cache = {}


def _slab_sizes(NV):
    """DMA slab sizes in [64,512] blocks: small first so compute starts
    early, then full SLAB-sized."""
    sizes = []
    for sz in (1, 1, 2, 4):
        if sum(sizes) >= NV:
            break
        sizes.append(min(sz, NV - sum(sizes)))
    while sum(sizes) < NV:
        sizes.append(min(SLAB, NV - sum(sizes)))
    return sizes


def _build_nc(CT):
    import concourse.bass as bass
    import concourse.mybir as mybir
    from concourse import tile

    f32 = mybir.dt.float32
    f32r = mybir.dt.float32r
    NV = CT // 2                     # [64, 512] 2-tile blocks
    LB = -(-CT // LBLK)              # l2 blocks
    sizes = _slab_sizes(NV)
    NS = len(sizes)
    slab_first_tile = []
    acc = 0
    for sz in sizes:
        slab_first_tile.append(2 * acc)
        acc += sz
    slab_off = [sum(sizes[:w]) for w in range(NS)]
    nc = bass.Bass()

    xs2 = nc.dram_tensor("xs2", [64, NV * TILE], f32, kind="ExternalInput")
    w0st = nc.dram_tensor("w0st", [64, 128], f32, kind="ExternalInput")
    w1bd = nc.dram_tensor("w1bd", [128, 128], f32, kind="ExternalInput")
    w2bk = nc.dram_tensor("w2bk", [128, LBLK * 64], f32, kind="ExternalInput")
    b0bd = nc.dram_tensor("b0bd", [128, 1], f32, kind="ExternalInput")
    b1bd = nc.dram_tensor("b1bd", [128, 1], f32, kind="ExternalInput")
    out_c = nc.dram_tensor("out_c", [64, LB * TILE], f32, kind="ExternalOutput")

    with tile.TileContext(nc) as tc:
        with (
            tc.tile_pool(name="const", bufs=1) as cpool,
            tc.tile_pool(name="xt", bufs=4) as xtpool,
            tc.tile_pool(name="h0sb", bufs=3) as h0pool,
            tc.tile_pool(name="h1sb", bufs=LAG + 3) as h1pool,
            tc.tile_pool(name="outsb", bufs=1) as opool,
            tc.tile_pool(name="ps_h0", bufs=3, space="PSUM") as ps_h0,
            tc.tile_pool(name="ps_h1", bufs=2, space="PSUM") as ps_h1,
            tc.tile_pool(name="ps_l2", bufs=1, space="PSUM") as ps_l2,
        ):
            # consts via SP/HWDGE as f32 (gpsimd SWDGE is reserved for the
            # xs slabs; only gpsimd DMAs may cast), f32r conversion on the
            # idle DVE
            w0f = cpool.tile([64, 128], f32, tag="w0f")
            nc.sync.dma_start(w0f[:], w0st[:])
            b0_sb = cpool.tile([128, 1], f32, tag="b0")
            nc.sync.dma_start(b0_sb[:], b0bd[:])
            w1f = cpool.tile([128, 128], f32, tag="w1f")
            nc.sync.dma_start(w1f[:], w1bd[:])
            b1_sb = cpool.tile([128, 1], f32, tag="b1")
            nc.sync.dma_start(b1_sb[:], b1bd[:])
            w0_sb = cpool.tile([64, 128], f32r, tag="w0")
            nc.vector.tensor_copy(w0_sb[:], w0f[:])
            w1_sb = cpool.tile([128, 128], f32r, tag="w1")
            nc.vector.tensor_copy(w1_sb[:], w1f[:])
            # loaded mid-loop (between slab3 and slab4 on the Pool queue) so
            # its 2.9us transfer never delays early xs slabs; the l2 LAG
            # covers its late arrival
            w2_sb = cpool.tile([128, LBLK * 64], f32r, tag="w2")

            out_sb = opool.tile([64, LB * TILE], f32, tag="out")

            slabs = [None] * NS

            def issue_slab(w):
                if w >= NS or slabs[w] is not None:
                    return
                bs = sizes[w]
                slabs[w] = xtpool.tile(
                    [64, SLAB * TILE], f32r, tag="xt", name="slab")
                nc.gpsimd.dma_start(
                    slabs[w][:, : bs * TILE],
                    xs2[:, TILE * slab_off[w] : TILE * (slab_off[w] + bs)],
                )

            for w in range(min(3, NS)):
                issue_slab(w)

            def emit_l2(pr):
                """L2 matmuls for tile pair pr (tiles 2pr, 2pr+1), plus the
                block evac + out-chunk DMA at block boundaries."""
                h1, _ = pending_h1[pr]
                for tt in (2 * pr, 2 * pr + 1):
                    b, j = divmod(tt, LBLK)
                    if j == 0:
                        l2state[0] = ps_l2.tile(
                            [64, TILE], f32, tag="l2", name="l2ps")
                    last = j == LBLK - 1 or tt == CT - 1
                    nc.tensor.matmul(
                        l2state[0][:], w2_sb[:, 64 * j : 64 * (j + 1)],
                        h1[:, TILE * (tt % 2) : TILE * (tt % 2 + 1)],
                        start=(j == 0), stop=last,
                    )
                    if last:
                        nc.scalar.copy(
                            out_sb[:, TILE * b : TILE * (b + 1)], l2state[0][:]
                        )
                        nc.sync.dma_start(
                            out_c[:, TILE * b : TILE * (b + 1)],
                            out_sb[:, TILE * b : TILE * (b + 1)],
                        )

            pending_h1 = {}
            l2state = [None]
            state = dict(h1ps=None, ndone=0)

            def emit_l1(tq, h0q):
                """L1 matmul for tile tq (lagged 2 tiles behind L0 so PE
                never waits on ACT's h0-evac latency), h1-pair evac on DVE,
                and the lagged l2 drain."""
                p = tq % 2
                if p == 0:
                    state["h1ps"] = ps_h1.tile(
                        [128, 2 * TILE], f32, tag="h1ps", name="h1ps")
                nc.tensor.matmul(
                    state["h1ps"][:, TILE * p : TILE * (p + 1)], w1_sb[:],
                    h0q[:], start=True, stop=True,
                )
                if p == 1:
                    h1 = h1pool.tile([128, 2 * TILE], f32r, tag="h1", name="h1")
                    nc.vector.tensor_scalar(
                        h1[:], state["h1ps"][:], b1_sb[:, 0:1], 0.0,
                        mybir.AluOpType.add, mybir.AluOpType.max,
                    )
                    pr = tq // 2
                    pending_h1[pr] = (h1, tq)
                    # lag l2 so the (late-loaded) w2 stationary never stalls
                    # PE's in-order stream; catch up to a steady 2-pair lag
                    # (keeps L0/L1 work between an l2 block's stop-matmul and
                    # the next block's start, hiding the evac WAR stall)
                    target = max(2, LAG - max(0, pr - LAG - 3))
                    while pending_h1 and min(pending_h1) <= pr - target:
                        pq = min(pending_h1)
                        emit_l2(pq)
                        del pending_h1[pq]

            pending_l1 = []
            wslab = 0
            for t in range(CT):
                v, s = divmod(t, 2)
                if v >= slab_off[wslab] + sizes[wslab]:
                    wslab += 1
                vv = v - slab_off[wslab]
                if s == 0 and vv == 0:
                    issue_slab(wslab + 3)
                if t == min(1, CT - 1):
                    nc.gpsimd.dma_start(w2_sb[:], w2bk[:])
                slab = slabs[wslab]
                h0ps = ps_h0.tile([128, TILE], f32, tag="h0ps")
                nc.tensor.matmul(
                    h0ps[:], w0_sb[32 * s : 32 * (s + 1), :],
                    slab[32 * s : 32 * (s + 1), TILE * vv : TILE * (vv + 1)],
                    start=True, stop=True,
                )
                h0 = h0pool.tile([128, TILE], f32r, tag="h0")
                nc.scalar.activation(
                    h0[:], h0ps[:], mybir.ActivationFunctionType.Relu,
                    bias=b0_sb[:, 0:1],
                )
                pending_l1.append((t, h0))
                if len(pending_l1) > 2:
                    emit_l1(*pending_l1.pop(0))
            for tq, h0q in pending_l1:
                emit_l1(tq, h0q)
            for pq in sorted(pending_h1):
                emit_l2(pq)

    _split_ctrl_waits(nc, mybir)
    return nc


def _split_ctrl_waits(nc, mybir):
    """walrus in this container accepts only one sync-wait per instruction;
    Tile attaches one wait per dependency lane. Hoist extras onto preceding
    single-wait nops on the same engine (equivalent ordering semantics)."""
    for bb in nc.main_func.blocks:
        newlist = []
        changed = False
        for ins in bb.instructions:
            si = ins.sync_info
            if si is not None and len(si.on_wait) > 1:
                waits = list(si.on_wait)
                for j, w in enumerate(waits[:-1]):
                    nop = mybir.InstNoOp(name=f"{ins.name}-wsplit-{j}", ins=[], outs=[])
                    nop.engine = ins.engine
                    nop.sync_info = mybir.SyncInfo(on_wait=[w], on_update=[])
                    newlist.append(nop)
                si.on_wait = [waits[-1]]
                ins.sync_info = si
                changed = True
            newlist.append(ins)
        if changed:
            bb.instructions = newlist
    return nc


def _prep_core_consts(W0k, b0k, W1k, b1k, W2k):
    f = np.float32
    w0bd = np.zeros((32, 128), f)
    w0bd[:16, :64] = W0k
    w0bd[16:, 64:] = W0k
    w0st = np.tile(w0bd, (2, 1))                      # [64, 128]
    w1 = np.zeros((128, 128), f)
    w1[:64, :64] = W1k
    w1[64:, 64:] = W1k
    w2bk = np.zeros((128, LBLK * 64), f)
    for j in range(LBLK):
        w2bk[:64, 64 * j + 2 * j] = W2k[:, 0]
        w2bk[64:, 64 * j + 2 * j + 1] = W2k[:, 0]
    b0 = np.concatenate([b0k, b0k]).astype(f).reshape(128, 1)
    b1 = np.concatenate([b1k, b1k]).astype(f).reshape(128, 1)
    return dict(w0st=w0st, w1bd=w1, w2bk=w2bk, b0bd=b0, b1bd=b1)


def _pack_xs(xs_k, CT):
    """[count, 16] -> [64, (CT/2)*512]; see module docstring."""
    NV = CT // 2
    X = np.zeros((CT * 1024, D_IN), np.float32)
    X[: len(xs_k)] = xs_k
    A = X.reshape(NV, 2, TILE, 2, D_IN)               # [v, s, c, o, f]
    return np.ascontiguousarray(
        A.transpose(1, 3, 4, 0, 2).reshape(64, NV * TILE))


def kernel(idxs, xs, W0, b0, W1, b1, W2, b2):
    from concourse.bass_utils import run_bass_kernel_spmd

    idx_flat = np.asarray(idxs).reshape(N)
    xs_flat = np.ascontiguousarray(np.asarray(xs, np.float32).reshape(N, D_IN))
    W0, b0 = np.asarray(W0, np.float32), np.asarray(b0, np.float32)
    W1, b1 = np.asarray(W1, np.float32), np.asarray(b1, np.float32)
    W2, b2 = np.asarray(W2, np.float32), np.asarray(b2, np.float32)

    order = np.argsort(idx_flat, kind="stable")
    counts = np.bincount(idx_flat, minlength=K)
    starts = np.zeros(K + 1, np.int64)
    starts[1:] = np.cumsum(counts)

    CT = max(2, -(-int(counts.max()) // 1024))
    CT = -(-CT // 2) * 2                              # multiple of 2
    if CT not in _cache:
        _cache[CT] = _build_nc(CT)
        _cache["nc"] = _cache[CT]                     # for test.py's TimelineSim
    nc = _cache[CT]
    LB = -(-CT // LBLK)

    in_maps = []
    perms = []
    for c in range(NCORES):
        perm_k = order[starts[c] : starts[c + 1]]
        perms.append(perm_k)
        in_maps.append(dict(
            xs2=_pack_xs(xs_flat[perm_k], CT),
            **_prep_core_consts(W0[c], b0[c], W1[c], b1[c], W2[c]),
        ))

    res = run_bass_kernel_spmd(nc, in_maps, list(range(NCORES))).results
    out = np.empty(N, np.float32)
    for c in range(NCORES):
        oc = res[c]["out_c"].reshape(32, 2, LB, TILE)
        vals = oc.transpose(2, 0, 3, 1).reshape(-1)[: counts[c]]
        out[perms[c]] = vals + b2[c, 0]
    return out.reshape(R, S, 1)


# revision 28
# speedup vs baseline: 1.1685x; 1.0263x over previous
"""MultiPropMLP (MoE-routed tiny MLP) Trainium2 kernel — expert-sharded.

Problem: out[n] = MLP_{idx[n]}(xs[n]) for N = 8192*128 samples, K = 8 experts,
MLP = 16 -> 64 -> relu -> 64 -> relu -> 1 with per-expert weights.

Sharding: expert-parallel across the 8 NeuronCores (K == n_cores). The host
routes each sample to the core owning its expert (stable argsort of idxs, a
pure sharding/layout step), so every core runs ONE dense 3-layer MLP over
~N/8 samples — no on-device routing, masking, gather, or idx tensor at all.
This removes the 8x all-K overcompute of the data-parallel formulation (the
previous 982us kernel): PE work drops from ~12 to 1.5 cycles/sample and the
PSUM-evacuation volume drops 8x.

Per-core layout (host-packed, feature-major, 2 samples per matmul column):
  tile t (1024 samples) = xt [32, 512]: col c holds sample 2c in partitions
  0-15 (features) and sample 2c+1 in partitions 16-31. Tiles are stacked 2
  deep across partitions (matmul operand base partitions are limited to
  {0,32,64}) -> DRAM xs2 [64, (CT/2)*512]; one [64, 8*512] slab DMA (gpsimd,
  the only engine allowed to cast f32->f32r) feeds 16 tiles. Per tile (all
  matmuls float32r, 512-col moving => 1 cyc/row):
    h0 [128,512] = relu(w0bd.T @ xt + b0)    w0bd = diag(W0, W0) [32, 128]
                                             (replicated at partitions
                                             0/32/64/96 to satisfy the
                                             lhsT/rhs same-base-partition rule)
    h1 [128,512] = relu(w1bd.T @ h0 + b1)    w1bd = diag(W1, W1) [128, 128]
    l2 [64, 512] += w2s_j.T @ h1             w2s_j [128, 64]: zero except col
                                             2j (rows :64) and 2j+1 (rows 64:)
                                             = W2, j = t % 32: 32 tiles
                                             accumulate into one PSUM block,
                                             amortizing the tiny-output evac.
  Engine balance per tile: PE 3x213ns; ACT h0 evac (relu+bias, 612ns); DVE
  h1 evac as [128,1024] pairs (596ns/tile); l2 block copy on DVE every 32
  tiles. b2 is folded in on the host during unpermute.

Note: walrus in this toolchain accepts only ONE sync-wait per instruction;
_split_ctrl_waits() hoists Tile's multi-waits onto single-wait nops.
"""

import numpy as np

R, S, D_IN, WIDTH, K = 8192, 128, 16, 64, 8
N = R * S
NCORES = 8
TILE = 512          # moving columns per matmul tile (= 1024 samples)
LBLK = 32           # tiles accumulated per l2 PSUM block
SLAB = 8            # [64,512] blocks (16 tiles) per xs DMA slab
LAG = 8             # tile-pairs the l2 matmuls trail the L0/L1 stream by

_cache = {}


def _slab_sizes(NV):
    """DMA slab sizes in [64,512] blocks: small first so compute starts
    early, then full SLAB-sized."""
    sizes = []
    for sz in (1, 1, 2, 4):
        if sum(sizes) >= NV:
            break
        sizes.append(min(sz, NV - sum(sizes)))
    while sum(sizes) < NV:
        sizes.append(min(SLAB, NV - sum(sizes)))
    return sizes


def _build_nc(CT):
    import concourse.bass as bass
    import concourse.mybir as mybir
    from concourse import tile

    f32 = mybir.dt.float32
    f32r = mybir.dt.float32r
    NV = CT // 2                     # [64, 512] 2-tile blocks
    LB = -(-CT // LBLK)              # l2 blocks
    sizes = _slab_sizes(NV)
    NS = len(sizes)
    slab_first_tile = []
    acc = 0
    for sz in sizes:
        slab_first_tile.append(2 * acc)
        acc += sz
    slab_off = [sum(sizes[:w]) for w in range(NS)]
    nc = bass.Bass()

    xs2 = nc.dram_tensor("xs2", [64, NV * TILE], f32, kind="ExternalInput")
    w0st = nc.dram_tensor("w0st", [64, 128], f32, kind="ExternalInput")
    w1bd = nc.dram_tensor("w1bd", [128, 128], f32, kind="ExternalInput")
    w2bk = nc.dram_tensor("w2bk", [128, LBLK * 64], f32, kind="ExternalInput")
    b0bd = nc.dram_tensor("b0bd", [128, 1], f32, kind="ExternalInput")
    b1bd = nc.dram_tensor("b1bd", [128, 1], f32, kind="ExternalInput")
    out_c = nc.dram_tensor("out_c", [64, LB * TILE], f32, kind="ExternalOutput")

    with tile.TileContext(nc) as tc:
        with (
            tc.tile_pool(name="const", bufs=1) as cpool,
            tc.tile_pool(name="xt", bufs=4) as xtpool,
            tc.tile_pool(name="h0sb", bufs=4) as h0pool,
            tc.tile_pool(name="h1sb", bufs=LAG + 3) as h1pool,
            tc.tile_pool(name="outsb", bufs=1) as opool,
            tc.tile_pool(name="ps_h0", bufs=3, space="PSUM") as ps_h0,
            tc.tile_pool(name="ps_h1", bufs=2, space="PSUM") as ps_h1,
            tc.tile_pool(name="ps_l2", bufs=1, space="PSUM") as ps_l2,
        ):
            # consts via SP/HWDGE as f32 (gpsimd SWDGE is reserved for the
            # xs slabs; only gpsimd DMAs may cast), f32r conversion on the
            # idle DVE
            w0f = cpool.tile([64, 128], f32, tag="w0f")
            nc.sync.dma_start(w0f[:], w0st[:])
            b0_sb = cpool.tile([128, 1], f32, tag="b0")
            nc.sync.dma_start(b0_sb[:], b0bd[:])
            w1f = cpool.tile([128, 128], f32, tag="w1f")
            nc.sync.dma_start(w1f[:], w1bd[:])
            b1_sb = cpool.tile([128, 1], f32, tag="b1")
            nc.sync.dma_start(b1_sb[:], b1bd[:])
            w0_sb = cpool.tile([64, 128], f32r, tag="w0")
            nc.vector.tensor_copy(w0_sb[:], w0f[:])
            w1_sb = cpool.tile([128, 128], f32r, tag="w1")
            nc.vector.tensor_copy(w1_sb[:], w1f[:])
            # loaded mid-loop (between slab3 and slab4 on the Pool queue) so
            # its 2.9us transfer never delays early xs slabs; the l2 LAG
            # covers its late arrival
            w2_sb = cpool.tile([128, LBLK * 64], f32r, tag="w2")

            out_sb = opool.tile([64, LB * TILE], f32, tag="out")

            slabs = [None] * NS

            def issue_slab(w):
                if w >= NS or slabs[w] is not None:
                    return
                bs = sizes[w]
                slabs[w] = xtpool.tile(
                    [64, SLAB * TILE], f32r, tag="xt", name="slab")
                nc.gpsimd.dma_start(
                    slabs[w][:, : bs * TILE],
                    xs2[:, TILE * slab_off[w] : TILE * (slab_off[w] + bs)],
                )

            for w in range(min(3, NS)):
                issue_slab(w)

            def emit_l2(pr):
                """L2 matmuls for tile pair pr (tiles 2pr, 2pr+1), plus the
                block evac + out-chunk DMA at block boundaries."""
                h1, _ = pending_h1[pr]
                for tt in (2 * pr, 2 * pr + 1):
                    b, j = divmod(tt, LBLK)
                    if j == 0:
                        l2state[0] = ps_l2.tile(
                            [64, TILE], f32, tag="l2", name="l2ps")
                    last = j == LBLK - 1 or tt == CT - 1
                    nc.tensor.matmul(
                        l2state[0][:], w2_sb[:, 64 * j : 64 * (j + 1)],
                        h1[:, TILE * (tt % 2) : TILE * (tt % 2 + 1)],
                        start=(j == 0), stop=last,
                    )
                    if last:
                        nc.scalar.copy(
                            out_sb[:, TILE * b : TILE * (b + 1)], l2state[0][:]
                        )
                        nc.sync.dma_start(
                            out_c[:, TILE * b : TILE * (b + 1)],
                            out_sb[:, TILE * b : TILE * (b + 1)],
                        )

            pending_h1 = {}
            l2state = [None]
            state = dict(h1ps=None, ndone=0)

            def emit_l1(tq, h0q):
                """L1 matmul for tile tq (lagged 2 tiles behind L0 so PE
                never waits on ACT's h0-evac latency), h1-pair evac on DVE,
                and the lagged l2 drain."""
                p = tq % 2
                if p == 0:
                    state["h1ps"] = ps_h1.tile(
                        [128, 2 * TILE], f32, tag="h1ps", name="h1ps")
                nc.tensor.matmul(
                    state["h1ps"][:, TILE * p : TILE * (p + 1)], w1_sb[:],
                    h0q[:], start=True, stop=True,
                )
                if p == 1:
                    h1 = h1pool.tile([128, 2 * TILE], f32r, tag="h1", name="h1")
                    nc.vector.tensor_scalar(
                        h1[:], state["h1ps"][:], b1_sb[:, 0:1], 0.0,
                        mybir.AluOpType.add, mybir.AluOpType.max,
                    )
                    pr = tq // 2
                    pending_h1[pr] = (h1, tq)
                    # lag l2 so the (late-loaded) w2 stationary never stalls
                    # PE's in-order stream; catch up to a steady 2-pair lag
                    # (keeps L0/L1 work between an l2 block's stop-matmul and
                    # the next block's start, hiding the evac WAR stall)
                    target = max(2, LAG - max(0, pr - LAG - 3))
                    while pending_h1 and min(pending_h1) <= pr - target:
                        pq = min(pending_h1)
                        emit_l2(pq)
                        del pending_h1[pq]

            pending_l1 = []
            wslab = 0
            for t in range(CT):
                v, s = divmod(t, 2)
                if v >= slab_off[wslab] + sizes[wslab]:
                    wslab += 1
                vv = v - slab_off[wslab]
                if s == 0 and vv == 0:
                    issue_slab(wslab + 3)
                if t == min(1, CT - 1):
                    nc.gpsimd.dma_start(w2_sb[:], w2bk[:])
                slab = slabs[wslab]
                h0ps = ps_h0.tile([128, TILE], f32, tag="h0ps")
                nc.tensor.matmul(
                    h0ps[:], w0_sb[32 * s : 32 * (s + 1), :],
                    slab[32 * s : 32 * (s + 1), TILE * vv : TILE * (vv + 1)],
                    start=True, stop=True,
                )
                h0 = h0pool.tile([128, TILE], f32r, tag="h0")
                nc.scalar.activation(
                    h0[:], h0ps[:], mybir.ActivationFunctionType.Relu,
                    bias=b0_sb[:, 0:1],
                )
                pending_l1.append((t, h0))
                if len(pending_l1) > 2:
                    emit_l1(*pending_l1.pop(0))
            for tq, h0q in pending_l1:
                emit_l1(tq, h0q)
            for pq in sorted(pending_h1):
                emit_l2(pq)

    _split_ctrl_waits(nc, mybir)
    return nc


def _split_ctrl_waits(nc, mybir):
    """walrus in this container accepts only one sync-wait per instruction;
    Tile attaches one wait per dependency lane. Hoist extras onto preceding
    single-wait nops on the same engine (equivalent ordering semantics)."""
    for bb in nc.main_func.blocks:
        newlist = []
        changed = False
        for ins in bb.instructions:
            si = ins.sync_info
            if si is not None and len(si.on_wait) > 1:
                waits = list(si.on_wait)
                for j, w in enumerate(waits[:-1]):
                    nop = mybir.InstNoOp(name=f"{ins.name}-wsplit-{j}", ins=[], outs=[])
                    nop.engine = ins.engine
                    nop.sync_info = mybir.SyncInfo(on_wait=[w], on_update=[])
                    newlist.append(nop)
                si.on_wait = [waits[-1]]
                ins.sync_info = si
                changed = True
            newlist.append(ins)
        if changed:
            bb.instructions = newlist
    return nc


def _prep_core_consts(W0k, b0k, W1k, b1k, W2k):
    f = np.float32
    w0bd = np.zeros((32, 128), f)
    w0bd[:16, :64] = W0k
    w0bd[16:, 64:] = W0k
    w0st = np.tile(w0bd, (2, 1))                      # [64, 128]
    w1 = np.zeros((128, 128), f)
    w1[:64, :64] = W1k
    w1[64:, 64:] = W1k
    w2bk = np.zeros((128, LBLK * 64), f)
    for j in range(LBLK):
        w2bk[:64, 64 * j + 2 * j] = W2k[:, 0]
        w2bk[64:, 64 * j + 2 * j + 1] = W2k[:, 0]
    b0 = np.concatenate([b0k, b0k]).astype(f).reshape(128, 1)
    b1 = np.concatenate([b1k, b1k]).astype(f).reshape(128, 1)
    return dict(w0st=w0st, w1bd=w1, w2bk=w2bk, b0bd=b0, b1bd=b1)


def _pack_xs(xs_k, CT):
    """[count, 16] -> [64, (CT/2)*512]; see module docstring."""
    NV = CT // 2
    X = np.zeros((CT * 1024, D_IN), np.float32)
    X[: len(xs_k)] = xs_k
    A = X.reshape(NV, 2, TILE, 2, D_IN)               # [v, s, c, o, f]
    return np.ascontiguousarray(
        A.transpose(1, 3, 4, 0, 2).reshape(64, NV * TILE))


def kernel(idxs, xs, W0, b0, W1, b1, W2, b2):
    from concourse.bass_utils import run_bass_kernel_spmd

    idx_flat = np.asarray(idxs).reshape(N)
    xs_flat = np.ascontiguousarray(np.asarray(xs, np.float32).reshape(N, D_IN))
    W0, b0 = np.asarray(W0, np.float32), np.asarray(b0, np.float32)
    W1, b1 = np.asarray(W1, np.float32), np.asarray(b1, np.float32)
    W2, b2 = np.asarray(W2, np.float32), np.asarray(b2, np.float32)

    order = np.argsort(idx_flat, kind="stable")
    counts = np.bincount(idx_flat, minlength=K)
    starts = np.zeros(K + 1, np.int64)
    starts[1:] = np.cumsum(counts)

    CT = max(2, -(-int(counts.max()) // 1024))
    CT = -(-CT // 2) * 2                              # multiple of 2
    if CT not in _cache:
        _cache[CT] = _build_nc(CT)
        _cache["nc"] = _cache[CT]                     # for test.py's TimelineSim
    nc = _cache[CT]
    LB = -(-CT // LBLK)

    in_maps = []
    perms = []
    for c in range(NCORES):
        perm_k = order[starts[c] : starts[c + 1]]
        perms.append(perm_k)
        in_maps.append(dict(
            xs2=_pack_xs(xs_flat[perm_k], CT),
            **_prep_core_consts(W0[c], b0[c], W1[c], b1[c], W2[c]),
        ))

    res = run_bass_kernel_spmd(nc, in_maps, list(range(NCORES))).results
    out = np.empty(N, np.float32)
    for c in range(NCORES):
        oc = res[c]["out_c"].reshape(32, 2, LB, TILE)
        vals = oc.transpose(2, 0, 3, 1).reshape(-1)[: counts[c]]
        out[perms[c]] = vals + b2[c, 0]
    return out.reshape(R, S, 1)


# revision 46
# speedup vs baseline: 1.1855x; 1.0145x over previous
"""MultiPropMLP (MoE-routed tiny MLP) Trainium2 kernel — expert-sharded.

Problem: out[n] = MLP_{idx[n]}(xs[n]) for N = 8192*128 samples, K = 8 experts,
MLP = 16 -> 64 -> relu -> 64 -> relu -> 1 with per-expert weights.

Sharding: expert-parallel across the 8 NeuronCores (K == n_cores). The host
routes each sample to the core owning its expert (stable argsort of idxs, a
pure sharding/layout step), so every core runs ONE dense 3-layer MLP over
~N/8 samples — no on-device routing, masking, gather, or idx tensor at all.
This removes the 8x all-K overcompute of the data-parallel formulation (the
previous 982us kernel): PE work drops from ~12 to 1.5 cycles/sample and the
PSUM-evacuation volume drops 8x.

Per-core layout (host-packed, feature-major, 2 samples per matmul column):
  tile t (1024 samples) = xt [32, 512]: col c holds sample 2c in partitions
  0-15 (features) and sample 2c+1 in partitions 16-31. Tiles are stacked 2
  deep across partitions (matmul operand base partitions are limited to
  {0,32,64}) -> DRAM xs2 [64, (CT/2)*512]; slab DMAs (gpsimd, the only
  engine allowed to cast f32->f32r) feed up to 8 tiles each, ramped
  1,1,2,4-block so compute starts ~4us in. Per tile (all matmuls float32r,
  512-col moving => 1 cyc/row at the PE's full 2.4GHz):
    h0 [128,512] = relu(w0st.T @ xt + b0)    w0st = 2x diag(W0, W0) [64, 128]
                                             (stacked so lhsT/rhs share a
                                             base partition, 0 or 32)
    h1 [128,512] = relu(w1bd.T @ h0 + b1)    w1bd = diag(W1, W1) [128, 128]
    l2 [64, 512] += w2s_j.T @ h1             w2s_j [128, 64]: zero except col
                                             2j (rows :64) and 2j+1 (rows 64:)
                                             = W2, j = t % 32: 32 tiles
                                             accumulate into one PSUM block,
                                             amortizing the tiny-output evac.
  Engine balance per tile: PE 3x213ns (fully packed); ACT h0 evac
  (relu+bias, 612ns) + l2 block copies; DVE h1 evac as [128,1024] pairs
  (596ns/tile). Two software-pipelining lags keep PE's in-order stream
  stall-free: L1 trails L0 by 2 tiles (hides ACT evac latency) and l2
  trails L1 by LAG pairs at first (hides the late gpsimd load of the big
  w2 stationary, issued behind the first xs slabs) easing to STEADY=2
  (hides the l2 PSUM WAR stall at block boundaries, ps_l2 bufs=1). Output
  blocks DMA out as they finish; b2 is folded in on the host during
  unpermute. ~95us/core on the TimelineSim cost model vs 982us for the
  dense all-K data-parallel formulation (PE 84.3us busy = 87%).

Note: walrus in this toolchain accepts only ONE sync-wait per instruction;
_split_ctrl_waits() hoists Tile's multi-waits onto single-wait nops.
"""

import numpy as np

R, S, D_IN, WIDTH, K = 8192, 128, 16, 64, 8
N = R * S
NCORES = 8
TILE = 512          # moving columns per matmul tile (= 1024 samples)
LBLK = 32           # tiles accumulated per l2 PSUM block
SLAB = 4            # [64,512] blocks (8 tiles) per xs DMA slab
LAG = 11            # tile-pairs the l2 matmuls trail the L0/L1 stream by
STEADY = 2          # steady-state l2 lag in pairs
XTBUFS = 4          # xs slab tile buffers (prefetch depth = XTBUFS-1)

_cache = {}


RAMP = (1, 1, 2, 4)


def _slab_sizes(NV):
    """DMA slab sizes in [64,512] blocks: small first so compute starts
    early, then full SLAB-sized."""
    sizes = []
    for sz in RAMP:
        if sum(sizes) >= NV:
            break
        sizes.append(min(sz, NV - sum(sizes)))
    while sum(sizes) < NV:
        sizes.append(min(SLAB, NV - sum(sizes)))
    return sizes


def _build_nc(CT):
    import concourse.bass as bass
    import concourse.mybir as mybir
    from concourse import tile

    f32 = mybir.dt.float32
    f32r = mybir.dt.float32r
    NV = CT // 2                     # [64, 512] 2-tile blocks
    LB = -(-CT // LBLK)              # l2 blocks
    sizes = _slab_sizes(NV)
    NS = len(sizes)
    slab_off = [sum(sizes[:w]) for w in range(NS)]
    nc = bass.Bass()

    xs2 = nc.dram_tensor("xs2", [64, NV * TILE], f32, kind="ExternalInput")
    w0st = nc.dram_tensor("w0st", [64, 128], f32, kind="ExternalInput")
    w1bd = nc.dram_tensor("w1bd", [128, 128], f32, kind="ExternalInput")
    w2bk = nc.dram_tensor("w2bk", [128, LBLK * 64], f32, kind="ExternalInput")
    b0bd = nc.dram_tensor("b0bd", [128, 1], f32, kind="ExternalInput")
    b1bd = nc.dram_tensor("b1bd", [128, 1], f32, kind="ExternalInput")
    out_c = nc.dram_tensor("out_c", [64, LB * TILE], f32, kind="ExternalOutput")

    with tile.TileContext(nc) as tc:
        with (
            tc.tile_pool(name="const", bufs=1) as cpool,
            tc.tile_pool(name="xt", bufs=XTBUFS) as xtpool,
            tc.tile_pool(name="h0sb", bufs=4) as h0pool,
            tc.tile_pool(name="h1sb", bufs=LAG + 3) as h1pool,
            tc.tile_pool(name="outsb", bufs=1) as opool,
            tc.tile_pool(name="ps_h0", bufs=3, space="PSUM") as ps_h0,
            tc.tile_pool(name="ps_h1", bufs=2, space="PSUM") as ps_h1,
            tc.tile_pool(name="ps_l2", bufs=1, space="PSUM") as ps_l2,
        ):
            # consts via SP/HWDGE as f32 (gpsimd SWDGE is reserved for the
            # xs slabs; only gpsimd DMAs may cast), f32r conversion on the
            # idle DVE
            w0f = cpool.tile([64, 128], f32, tag="w0f")
            nc.sync.dma_start(w0f[:], w0st[:])
            b0_sb = cpool.tile([128, 1], f32, tag="b0")
            nc.sync.dma_start(b0_sb[:], b0bd[:])
            w1f = cpool.tile([128, 128], f32, tag="w1f")
            nc.sync.dma_start(w1f[:], w1bd[:])
            b1_sb = cpool.tile([128, 1], f32, tag="b1")
            nc.sync.dma_start(b1_sb[:], b1bd[:])
            w0_sb = cpool.tile([64, 128], f32r, tag="w0")
            nc.vector.tensor_copy(w0_sb[:], w0f[:])
            w1_sb = cpool.tile([128, 128], f32r, tag="w1")
            nc.vector.tensor_copy(w1_sb[:], w1f[:])
            # loaded mid-loop (between slab3 and slab4 on the Pool queue) so
            # its 2.9us transfer never delays early xs slabs; the l2 LAG
            # covers its late arrival
            w2_sb = cpool.tile([128, LBLK * 64], f32r, tag="w2")

            out_sb = opool.tile([64, LB * TILE], f32, tag="out")

            slabs = [None] * NS

            def issue_slab(w):
                if w >= NS or slabs[w] is not None:
                    return
                bs = sizes[w]
                slabs[w] = xtpool.tile(
                    [64, SLAB * TILE], f32r, tag="xt", name="slab")
                nc.gpsimd.dma_start(
                    slabs[w][:, : bs * TILE],
                    xs2[:, TILE * slab_off[w] : TILE * (slab_off[w] + bs)],
                )

            for w in range(min(XTBUFS - 1, NS)):
                issue_slab(w)

            def emit_l2(pr):
                """L2 matmuls for tile pair pr (tiles 2pr, 2pr+1), plus the
                block evac + out-chunk DMA at block boundaries."""
                h1, _ = pending_h1[pr]
                for tt in (2 * pr, 2 * pr + 1):
                    b, j = divmod(tt, LBLK)
                    if j == 0:
                        l2state[0] = ps_l2.tile(
                            [64, TILE], f32, tag="l2", name="l2ps")
                    last = j == LBLK - 1 or tt == CT - 1
                    nc.tensor.matmul(
                        l2state[0][:], w2_sb[:, 64 * j : 64 * (j + 1)],
                        h1[:, TILE * (tt % 2) : TILE * (tt % 2 + 1)],
                        start=(j == 0), stop=last,
                    )
                    if last:
                        rows = 2 * (min(CT, (b + 1) * LBLK) - b * LBLK)
                        nc.scalar.copy(
                            out_sb[:, TILE * b : TILE * (b + 1)], l2state[0][:]
                        )
                        nc.sync.dma_start(
                            out_c[:rows, TILE * b : TILE * (b + 1)],
                            out_sb[:rows, TILE * b : TILE * (b + 1)],
                        )

            pending_h1 = {}
            l2state = [None]
            state = {"h1ps": None}

            def emit_l1(tq, h0q):
                """L1 matmul for tile tq (lagged 2 tiles behind L0 so PE
                never waits on ACT's h0-evac latency), h1-pair evac on DVE,
                and the lagged l2 drain."""
                p = tq % 2
                if p == 0:
                    state["h1ps"] = ps_h1.tile(
                        [128, 2 * TILE], f32, tag="h1ps", name="h1ps")
                nc.tensor.matmul(
                    state["h1ps"][:, TILE * p : TILE * (p + 1)], w1_sb[:],
                    h0q[:], start=True, stop=True,
                )
                if p == 1:
                    h1 = h1pool.tile([128, 2 * TILE], f32r, tag="h1", name="h1")
                    nc.vector.tensor_scalar(
                        h1[:], state["h1ps"][:], b1_sb[:, 0:1], 0.0,
                        mybir.AluOpType.add, mybir.AluOpType.max,
                    )
                    pr = tq // 2
                    pending_h1[pr] = (h1, tq)
                    # lag l2 so the (late-loaded) w2 stationary never stalls
                    # PE's in-order stream; catch up to a steady 2-pair lag
                    # (keeps L0/L1 work between an l2 block's stop-matmul and
                    # the next block's start, hiding the evac WAR stall)
                    target = max(STEADY, LAG - max(0, pr - LAG - 3))
                    while pending_h1 and min(pending_h1) <= pr - target:
                        pq = min(pending_h1)
                        emit_l2(pq)
                        del pending_h1[pq]

            pending_l1 = []
            wslab = 0
            for t in range(CT):
                v, s = divmod(t, 2)
                if v >= slab_off[wslab] + sizes[wslab]:
                    wslab += 1
                vv = v - slab_off[wslab]
                if s == 0 and vv == 0:
                    issue_slab(wslab + XTBUFS - 1)
                if t == min(1, CT - 1):
                    nc.gpsimd.dma_start(w2_sb[:], w2bk[:])
                slab = slabs[wslab]
                h0ps = ps_h0.tile([128, TILE], f32, tag="h0ps")
                nc.tensor.matmul(
                    h0ps[:], w0_sb[32 * s : 32 * (s + 1), :],
                    slab[32 * s : 32 * (s + 1), TILE * vv : TILE * (vv + 1)],
                    start=True, stop=True,
                )
                h0 = h0pool.tile([128, TILE], f32r, tag="h0")
                nc.scalar.activation(
                    h0[:], h0ps[:], mybir.ActivationFunctionType.Relu,
                    bias=b0_sb[:, 0:1],
                )
                pending_l1.append((t, h0))
                if len(pending_l1) > 2:
                    emit_l1(*pending_l1.pop(0))
            # flush: interleave ready l2 pairs with the lagged L1s so l2
            # block-boundary WAR-waits overlap the remaining L1 work
            ready = sorted(pending_h1)
            while ready or pending_l1:
                if ready:
                    pq = ready.pop(0)
                    emit_l2(pq)
                    del pending_h1[pq]
                if pending_l1:
                    emit_l1(*pending_l1.pop(0))
            for pq in sorted(pending_h1):
                emit_l2(pq)

    _split_ctrl_waits(nc, mybir)
    return nc


def _split_ctrl_waits(nc, mybir):
    """walrus in this container accepts only one sync-wait per instruction;
    Tile attaches one wait per dependency lane. Hoist extras onto preceding
    single-wait nops on the same engine (equivalent ordering semantics)."""
    for bb in nc.main_func.blocks:
        newlist = []
        changed = False
        for ins in bb.instructions:
            si = ins.sync_info
            if si is not None and len(si.on_wait) > 1:
                waits = list(si.on_wait)
                for j, w in enumerate(waits[:-1]):
                    nop = mybir.InstNoOp(name=f"{ins.name}-wsplit-{j}", ins=[], outs=[])
                    nop.engine = ins.engine
                    nop.sync_info = mybir.SyncInfo(on_wait=[w], on_update=[])
                    newlist.append(nop)
                si.on_wait = [waits[-1]]
                ins.sync_info = si
                changed = True
            newlist.append(ins)
        if changed:
            bb.instructions = newlist
    return nc


def _prep_core_consts(W0k, b0k, W1k, b1k, W2k):
    f = np.float32
    w0bd = np.zeros((32, 128), f)
    w0bd[:16, :64] = W0k
    w0bd[16:, 64:] = W0k
    w0st = np.tile(w0bd, (2, 1))                      # [64, 128]
    w1 = np.zeros((128, 128), f)
    w1[:64, :64] = W1k
    w1[64:, 64:] = W1k
    w2bk = np.zeros((128, LBLK * 64), f)
    for j in range(LBLK):
        w2bk[:64, 64 * j + 2 * j] = W2k[:, 0]
        w2bk[64:, 64 * j + 2 * j + 1] = W2k[:, 0]
    b0 = np.concatenate([b0k, b0k]).astype(f).reshape(128, 1)
    b1 = np.concatenate([b1k, b1k]).astype(f).reshape(128, 1)
    return dict(w0st=w0st, w1bd=w1, w2bk=w2bk, b0bd=b0, b1bd=b1)


def _pack_xs(xs_k, CT):
    """[count, 16] -> [64, (CT/2)*512]; see module docstring."""
    NV = CT // 2
    X = np.zeros((CT * 1024, D_IN), np.float32)
    X[: len(xs_k)] = xs_k
    A = X.reshape(NV, 2, TILE, 2, D_IN)               # [v, s, c, o, f]
    return np.ascontiguousarray(
        A.transpose(1, 3, 4, 0, 2).reshape(64, NV * TILE))


def kernel(idxs, xs, W0, b0, W1, b1, W2, b2):
    from concourse.bass_utils import run_bass_kernel_spmd

    idx_flat = np.asarray(idxs).reshape(N)
    xs_flat = np.ascontiguousarray(np.asarray(xs, np.float32).reshape(N, D_IN))
    W0, b0 = np.asarray(W0, np.float32), np.asarray(b0, np.float32)
    W1, b1 = np.asarray(W1, np.float32), np.asarray(b1, np.float32)
    W2, b2 = np.asarray(W2, np.float32), np.asarray(b2, np.float32)

    order = np.argsort(idx_flat, kind="stable")
    counts = np.bincount(idx_flat, minlength=K)
    starts = np.zeros(K + 1, np.int64)
    starts[1:] = np.cumsum(counts)

    CT = max(2, -(-int(counts.max()) // 1024))
    CT = -(-CT // 2) * 2                              # multiple of 2
    if CT not in _cache:
        _cache[CT] = _build_nc(CT)
        _cache["nc"] = _cache[CT]                     # for test.py's TimelineSim
    nc = _cache[CT]
    LB = -(-CT // LBLK)

    in_maps = []
    perms = []
    for c in range(NCORES):
        perm_k = order[starts[c] : starts[c + 1]]
        perms.append(perm_k)
        in_maps.append(dict(
            xs2=_pack_xs(xs_flat[perm_k], CT),
            **_prep_core_consts(W0[c], b0[c], W1[c], b1[c], W2[c]),
        ))

    res = run_bass_kernel_spmd(nc, in_maps, list(range(NCORES))).results
    out = np.empty(N, np.float32)
    for c in range(NCORES):
        oc = res[c]["out_c"].reshape(32, 2, LB, TILE)
        vals = oc.transpose(2, 0, 3, 1).reshape(-1)[: counts[c]]
        out[perms[c]] = vals + b2[c, 0]
    return out.reshape(R, S, 1)



# revision 74
# speedup vs baseline: 1.2018x; 1.0138x over previous
"""MultiPropMLP (MoE-routed tiny MLP) Trainium2 kernel — expert-sharded.

Problem: out[n] = MLP_{idx[n]}(xs[n]) for N = 8192*128 samples, K = 8 experts,
MLP = 16 -> 64 -> relu -> 64 -> relu -> 1 with per-expert weights.

Sharding: expert-parallel across the 8 NeuronCores (K == n_cores). The host
routes each sample to the core owning its expert (stable argsort of idxs, a
pure sharding/layout step), so every core runs ONE dense 3-layer MLP over
~N/8 samples — no on-device routing, masking, gather, or idx tensor at all.
This removes the 8x all-K overcompute of the data-parallel formulation (the
previous 982us kernel): PE work drops from ~12 to 1.5 cycles/sample and the
PSUM-evacuation volume drops 8x.

Per-core layout (host-packed, feature-major, 2 samples per matmul column):
  tile t (1024 samples) = xt [32, 512]: col c holds sample 2c in partitions
  0-15 (features) and sample 2c+1 in partitions 16-31. Tiles are stacked 2
  deep across partitions (matmul operand base partitions are limited to
  {0,32,64}) -> DRAM xs2 [64, (CT/2)*512]; slab DMAs (gpsimd, the only
  engine allowed to cast f32->f32r) feed up to 8 tiles each, ramped
  1,1,2,4-block so compute starts ~4us in. Per tile (all matmuls float32r,
  512-col moving => 1 cyc/row at the PE's full 2.4GHz):
    h0 [128,512] = relu(w0st.T @ xt + b0)    w0st = 2x diag(W0, W0) [64, 128]
                                             (stacked so lhsT/rhs share a
                                             base partition, 0 or 32)
    h1 [128,512] = relu(w1bd.T @ h0 + b1)    w1bd = diag(W1, W1) [128, 128]
    l2 [64, 512] += w2s_j.T @ h1             w2s_j [128, 64]: zero except col
                                             2j (rows :64) and 2j+1 (rows 64:)
                                             = W2, j = t % 32: 32 tiles
                                             accumulate into one PSUM block,
                                             amortizing the tiny-output evac.
  Engine balance per tile: PE 3x213ns (fully packed); ACT h0 evac
  (relu+bias, 612ns) + l2 block copies; DVE h1 evac as [128,1024] pairs
  (596ns/tile). Two software-pipelining lags keep PE's in-order stream
  stall-free: L1 trails L0 by 2 tiles (hides ACT evac latency) and l2
  trails L1 by LAG pairs at first (hides the late gpsimd load of the big
  w2 stationary, issued behind the first xs slabs) easing to STEADY
  (hides the l2 PSUM WAR stall at block boundaries, ps_l2 bufs=1). The
  final tile may be partial (TW >= 256 moving columns), shortening the
  critical tail chain. Output blocks DMA out as they finish; b2 is folded
  in on the host during unpermute. ~93.8us/core on the TimelineSim cost
  model vs 982us for the dense all-K data-parallel formulation (PE ~83us
  busy = 88%).

Note: walrus in this toolchain accepts only ONE sync-wait per instruction;
_split_ctrl_waits() hoists Tile's multi-waits onto single-wait nops.
"""

import numpy as np

R, S, D_IN, WIDTH, K = 8192, 128, 16, 64, 8
N = R * S
NCORES = 8
TILE = 512          # moving columns per matmul tile (= 1024 samples)
LBLK = 32           # tiles accumulated per l2 PSUM block
SLAB = 4            # [64,512] blocks (8 tiles) per xs DMA slab
LAG = 11            # tile-pairs the l2 matmuls trail the L0/L1 stream by
STEADY = 5          # steady-state l2 lag in pairs
XTBUFS = 4          # xs slab tile buffers (prefetch depth = XTBUFS-1)

_cache = {}


RAMP = (1, 1, 2, 4)


def _slab_sizes(NV):
    """DMA slab sizes in [64,512] blocks: small first so compute starts
    early, then full SLAB-sized."""
    sizes = []
    for sz in RAMP:
        if sum(sizes) >= NV:
            break
        sizes.append(min(sz, NV - sum(sizes)))
    while sum(sizes) < NV:
        sizes.append(min(SLAB, NV - sum(sizes)))
    return sizes


def _build_nc(CT, TW=TILE):
    import concourse.bass as bass
    import concourse.mybir as mybir
    from concourse import tile

    def wt(t):
        """moving-column width of tile t (the final tile may be partial)"""
        return TW if t == CT - 1 else TILE

    f32 = mybir.dt.float32
    f32r = mybir.dt.float32r
    NV = (CT + 1) // 2               # [64, 512] 2-tile blocks (odd CT: the
                                     # last block's second tile is padding,
                                     # loaded but never computed)
    LB = -(-CT // LBLK)              # l2 blocks
    sizes = _slab_sizes(NV)
    NS = len(sizes)
    slab_off = [sum(sizes[:w]) for w in range(NS)]
    nc = bass.Bass()

    # xs2 cols 0:128 hold w0st (2x diag(W0,W0)); fused into slab0's casting
    # DMA so the first matmul's stationary needs no separate load+convert
    xs2 = nc.dram_tensor("xs2", [64, 128 + NV * TILE], f32, kind="ExternalInput")
    w1bd = nc.dram_tensor("w1bd", [128, 128], f32, kind="ExternalInput")
    w2bk = nc.dram_tensor("w2bk", [128, LBLK * 64], f32, kind="ExternalInput")
    b0bd = nc.dram_tensor("b0bd", [128, 1], f32, kind="ExternalInput")
    b1bd = nc.dram_tensor("b1bd", [128, 1], f32, kind="ExternalInput")
    out_c = nc.dram_tensor("out_c", [64, LB * TILE], f32, kind="ExternalOutput")

    with tile.TileContext(nc) as tc:
        with (
            tc.tile_pool(name="const", bufs=1) as cpool,
            tc.tile_pool(name="xt", bufs=XTBUFS) as xtpool,
            tc.tile_pool(name="h0sb", bufs=4) as h0pool,
            tc.tile_pool(name="h1sb", bufs=LAG + 3) as h1pool,
            tc.tile_pool(name="outsb", bufs=1) as opool,
            tc.tile_pool(name="ps_h0", bufs=3, space="PSUM") as ps_h0,
            tc.tile_pool(name="ps_h1", bufs=2, space="PSUM") as ps_h1,
            tc.tile_pool(name="ps_l2", bufs=1, space="PSUM") as ps_l2,
        ):
            # remaining consts via SP/HWDGE as f32 (gpsimd SWDGE is reserved
            # for the xs slabs; only gpsimd DMAs may cast), f32r conversion
            # on the idle DVE
            b0_sb = cpool.tile([128, 1], f32, tag="b0")
            nc.sync.dma_start(b0_sb[:], b0bd[:])
            w1f = cpool.tile([128, 128], f32, tag="w1f")
            nc.sync.dma_start(w1f[:], w1bd[:])
            b1_sb = cpool.tile([128, 1], f32, tag="b1")
            nc.sync.dma_start(b1_sb[:], b1bd[:])
            w1_sb = cpool.tile([128, 128], f32r, tag="w1")
            nc.vector.tensor_copy(w1_sb[:], w1f[:])
            # loaded mid-loop (between slab3 and slab4 on the Pool queue) so
            # its 2.9us transfer never delays early xs slabs; the l2 LAG
            # covers its late arrival
            w2_sb = cpool.tile([128, LBLK * 64], f32r, tag="w2")

            out_sb = opool.tile([64, LB * TILE], f32, tag="out")

            slabs = [None] * NS
            # slab0 rides with w0st in one casting DMA; lives in cpool so
            # the w0 stationary slice stays valid the whole run
            slab0w0 = cpool.tile([64, 128 + sizes[0] * TILE], f32r, tag="s0w0")
            nc.gpsimd.dma_start(
                slab0w0[:], xs2[:, : 128 + sizes[0] * TILE])
            slabs[0] = slab0w0

            def issue_slab(w):
                if w >= NS or slabs[w] is not None:
                    return
                bs = sizes[w]
                slabs[w] = xtpool.tile(
                    [64, SLAB * TILE], f32r, tag="xt", name="slab")
                nc.gpsimd.dma_start(
                    slabs[w][:, : bs * TILE],
                    xs2[:, 128 + TILE * slab_off[w] : 128 + TILE * (slab_off[w] + bs)],
                )

            for w in range(min(XTBUFS - 1, NS)):
                issue_slab(w)

            def emit_l2(pr):
                """L2 matmuls for tile pair pr (tiles 2pr, 2pr+1), plus the
                block evac + out-chunk DMA at block boundaries."""
                h1, _ = pending_h1[pr]
                for tt in (2 * pr, 2 * pr + 1):
                    if tt >= CT:
                        continue
                    b, j = divmod(tt, LBLK)
                    if j == 0:
                        l2state[0] = ps_l2.tile(
                            [64, TILE], f32, tag="l2", name="l2ps")
                    last = j == LBLK - 1 or tt == CT - 1
                    nc.tensor.matmul(
                        l2state[0][:, : wt(tt)], w2_sb[:, 64 * j : 64 * (j + 1)],
                        h1[:, TILE * (tt % 2) : TILE * (tt % 2) + wt(tt)],
                        start=(j == 0), stop=last,
                    )
                    if last:
                        rows = 2 * (min(CT, (b + 1) * LBLK) - b * LBLK)
                        # a lone partial final tile shrinks its whole block
                        bw = TW if b * LBLK == CT - 1 else TILE
                        nc.scalar.copy(
                            out_sb[:, TILE * b : TILE * b + bw],
                            l2state[0][:, :bw],
                        )
                        nc.sync.dma_start(
                            out_c[:rows, TILE * b : TILE * b + bw],
                            out_sb[:rows, TILE * b : TILE * b + bw],
                        )

            pending_h1 = {}
            l2state = [None]
            state = {"h1ps": None}

            def emit_l1(tq, h0q):
                """L1 matmul for tile tq (lagged 2 tiles behind L0 so PE
                never waits on ACT's h0-evac latency), h1-pair evac on DVE,
                and the lagged l2 drain."""
                p = tq % 2
                if p == 0:
                    state["h1ps"] = ps_h1.tile(
                        [128, 2 * TILE], f32, tag="h1ps", name="h1ps")
                nc.tensor.matmul(
                    state["h1ps"][:, TILE * p : TILE * p + wt(tq)], w1_sb[:],
                    h0q[:, : wt(tq)], start=True, stop=True,
                )
                if p == 1 or tq == CT - 1:
                    # evac the pair — or just the lone half for an odd CT's
                    # final singleton tile
                    ew = TILE * p + wt(tq)
                    h1 = h1pool.tile([128, 2 * TILE], f32r, tag="h1", name="h1")
                    nc.vector.tensor_scalar(
                        h1[:, :ew], state["h1ps"][:, :ew], b1_sb[:, 0:1], 0.0,
                        mybir.AluOpType.add, mybir.AluOpType.max,
                    )
                    pr = tq // 2
                    pending_h1[pr] = (h1, tq)
                    # lag l2 so the (late-loaded) w2 stationary never stalls
                    # PE's in-order stream; catch up to a steady 2-pair lag
                    # (keeps L0/L1 work between an l2 block's stop-matmul and
                    # the next block's start, hiding the evac WAR stall)
                    target = max(STEADY, LAG - max(0, pr - LAG - 3))
                    while pending_h1 and min(pending_h1) <= pr - target:
                        pq = min(pending_h1)
                        emit_l2(pq)
                        del pending_h1[pq]

            pending_l1 = []
            wslab = 0
            for t in range(CT):
                v, s = divmod(t, 2)
                if v >= slab_off[wslab] + sizes[wslab]:
                    wslab += 1
                vv = v - slab_off[wslab]
                if s == 0 and vv == 0:
                    issue_slab(wslab + XTBUFS - 1)
                if t == min(1, CT - 1):
                    nc.gpsimd.dma_start(w2_sb[:], w2bk[:])
                slab = slabs[wslab]
                cb = 128 if wslab == 0 else 0  # w0st cols in the fused tile
                h0ps = ps_h0.tile([128, TILE], f32, tag="h0ps")
                nc.tensor.matmul(
                    h0ps[:, : wt(t)], slab0w0[32 * s : 32 * (s + 1), 0:128],
                    slab[32 * s : 32 * (s + 1),
                         cb + TILE * vv : cb + TILE * vv + wt(t)],
                    start=True, stop=True,
                )
                h0 = h0pool.tile([128, TILE], f32r, tag="h0")
                nc.scalar.activation(
                    h0[:, : wt(t)], h0ps[:, : wt(t)],
                    mybir.ActivationFunctionType.Relu,
                    bias=b0_sb[:, 0:1],
                )
                pending_l1.append((t, h0))
                if len(pending_l1) > 2:
                    emit_l1(*pending_l1.pop(0))
            # flush: interleave ready l2 pairs with the lagged L1s so l2
            # block-boundary WAR-waits overlap the remaining L1 work
            ready = sorted(pending_h1)
            while ready or pending_l1:
                if ready:
                    pq = ready.pop(0)
                    emit_l2(pq)
                    del pending_h1[pq]
                if pending_l1:
                    emit_l1(*pending_l1.pop(0))
            for pq in sorted(pending_h1):
                emit_l2(pq)

    _split_ctrl_waits(nc, mybir)
    return nc


def _split_ctrl_waits(nc, mybir):
    """walrus in this container accepts only one sync-wait per instruction;
    Tile attaches one wait per dependency lane. Hoist extras onto preceding
    single-wait nops on the same engine (equivalent ordering semantics)."""
    for bb in nc.main_func.blocks:
        newlist = []
        changed = False
        for ins in bb.instructions:
            si = ins.sync_info
            if si is not None and len(si.on_wait) > 1:
                waits = list(si.on_wait)
                for j, w in enumerate(waits[:-1]):
                    nop = mybir.InstNoOp(name=f"{ins.name}-wsplit-{j}", ins=[], outs=[])
                    nop.engine = ins.engine
                    nop.sync_info = mybir.SyncInfo(on_wait=[w], on_update=[])
                    newlist.append(nop)
                si.on_wait = [waits[-1]]
                ins.sync_info = si
                changed = True
            newlist.append(ins)
        if changed:
            bb.instructions = newlist
    return nc


def _prep_core_consts(W0k, b0k, W1k, b1k, W2k):
    f = np.float32
    w0bd = np.zeros((32, 128), f)
    w0bd[:16, :64] = W0k
    w0bd[16:, 64:] = W0k
    w0st = np.tile(w0bd, (2, 1))                      # [64, 128]
    w1 = np.zeros((128, 128), f)
    w1[:64, :64] = W1k
    w1[64:, 64:] = W1k
    w2bk = np.zeros((128, LBLK * 64), f)
    for j in range(LBLK):
        w2bk[:64, 64 * j + 2 * j] = W2k[:, 0]
        w2bk[64:, 64 * j + 2 * j + 1] = W2k[:, 0]
    b0 = np.concatenate([b0k, b0k]).astype(f).reshape(128, 1)
    b1 = np.concatenate([b1k, b1k]).astype(f).reshape(128, 1)
    return dict(w0st=w0st, w1bd=w1, w2bk=w2bk, b0bd=b0, b1bd=b1)


def _pack_xs(xs_k, CT):
    """[count, 16] -> [64, ceil(CT/2)*512]; see module docstring."""
    NV = (CT + 1) // 2
    X = np.zeros((2 * NV * 1024, D_IN), np.float32)
    X[: len(xs_k)] = xs_k
    A = X.reshape(NV, 2, TILE, 2, D_IN)               # [v, s, c, o, f]
    return np.ascontiguousarray(
        A.transpose(1, 3, 4, 0, 2).reshape(64, NV * TILE))


def kernel(idxs, xs, W0, b0, W1, b1, W2, b2):
    from concourse.bass_utils import run_bass_kernel_spmd

    idx_flat = np.asarray(idxs).reshape(N)
    xs_flat = np.ascontiguousarray(np.asarray(xs, np.float32).reshape(N, D_IN))
    W0, b0 = np.asarray(W0, np.float32), np.asarray(b0, np.float32)
    W1, b1 = np.asarray(W1, np.float32), np.asarray(b1, np.float32)
    W2, b2 = np.asarray(W2, np.float32), np.asarray(b2, np.float32)

    order = np.argsort(idx_flat, kind="stable")
    counts = np.bincount(idx_flat, minlength=K)
    starts = np.zeros(K + 1, np.int64)
    starts[1:] = np.cumsum(counts)

    CT = max(2, -(-int(counts.max()) // 1024))
    # final tile may be partial (f32r needs >= 256 moving columns), but only
    # when it is alone in its l2 block so the PSUM accumulation group and
    # block evac stay uniform-width
    TW = max(256, min(TILE, -(-(int(counts.max()) - (CT - 1) * 1024) // 2)))
    if (CT - 1) % LBLK != 0:
        TW = TILE
    if (CT, TW) not in _cache:
        _cache[(CT, TW)] = _build_nc(CT, TW)
        _cache["nc"] = _cache[(CT, TW)]               # for test.py's TimelineSim
    nc = _cache[(CT, TW)]
    LB = -(-CT // LBLK)

    in_maps = []
    perms = []
    for c in range(NCORES):
        perm_k = order[starts[c] : starts[c + 1]]
        perms.append(perm_k)
        consts = _prep_core_consts(W0[c], b0[c], W1[c], b1[c], W2[c])
        w0st = consts.pop("w0st")
        in_maps.append(dict(
            xs2=np.ascontiguousarray(np.concatenate(
                [w0st, _pack_xs(xs_flat[perm_k], CT)], axis=1)),
            **consts,
        ))

    res = run_bass_kernel_spmd(nc, in_maps, list(range(NCORES))).results
    out = np.empty(N, np.float32)
    for c in range(NCORES):
        oc = res[c]["out_c"].reshape(32, 2, LB, TILE)
        vals = oc.transpose(2, 0, 3, 1).reshape(-1)[: counts[c]]
        out[perms[c]] = vals + b2[c, 0]
    return out.reshape(R, S, 1)

